# revision 55
# baseline (speedup 1.0000x reference)
"""Trainium2 Bass kernel for nn_Decoder — v3.

Changes vs v2 (247.5us):
- 2D sharding: 8 cores = 2 batch halves x 4 vocab shards. Recurrence stays
  8 rows/core; attn2 is AllGathered only within each half
  (replica_groups=[[0..3],[4..7]]), halving collective bytes. Each core
  computes V/4=8704 vocab rows for its half's 992 (t,b) columns.
- 3 collectives instead of 7 (15us fixed cost each, serialized on the
  collective engine, was the dominant cost).
- Vocab projection in fp8e4m3 DoubleRow (0.5 cyc/row vs 1.0 bf16) with a
  3-term residual expansion W8'A8 + Rw'A8 + W8'Ra (W8=fp8(16W),
  Rw=fp8(16W-W8), A8=fp8(A), Ra=fp8(A-A8)) — one PSUM scale (16x), host
  divides by 16. Measured 3.0e-3 rel err vs 2.6e-3 for bf16, at 3/4 the
  PE cycles.
- Score block reordered to (s,b) column order so keys+q runs on DVE in
  2x_2p mode with all-SBUF bf16 operands (658ns -> ~330ns), with q copied
  out of PSUM first.
- Per-step PE emission order: qT -> z(t+1)/att H-parts (ready early) ->
  score/softmax matmuls -> AL parts; vocab units pumped into the th-latency
  window.
"""

import numpy as np
import ml_dtypes

BF16 = ml_dtypes.bfloat16
F8 = ml_dtypes.float8_e4m3fn

B, T, S = 64, 31, 32
V, E, H, MDIM = 34004, 300, 512, 1024
NCORES = 8
GSIZE = 4                   # replica group size (batch half)
BS = B // NCORES            # 8 batch rows per core
CB = 4                      # batch rows per chain
VS = 8704                   # 68 x 128 vocab shard (V/4 padded)
V_PAD = VS * 4              # 34816
NMC = VS // 128             # 68
GROUPS = [(0, 3), (3, 12), (12, 19), (19, 26), (26, 31)]
READY_MS = [0.030, 0.060, 0.088, 0.115, 0.145]
RIPE = 0

_CACHE = {}


def _build_program():
    import concourse.bass as bass
    import concourse.bacc as bacc
    import concourse.mybir as mybir
    import concourse.tile as tile

    dt = mybir.dt
    f32 = dt.float32
    bf16 = dt.bfloat16
    fp8 = dt.float8e4
    AF = mybir.ActivationFunctionType
    OP = mybir.AluOpType
    DR = mybir.MatmulPerfMode.DoubleRow

    nc = bacc.Bacc(num_devices=NCORES)

    W1L_d = nc.dram_tensor("W1L", [128, 4, 16, 128], bf16, kind="ExternalInput")
    M2L_d = nc.dram_tensor("M2L", [128, 2, 16, 128], bf16, kind="ExternalInput")
    M3L_d = nc.dram_tensor("M3L", [128, 2, 4, 128], bf16, kind="ExternalInput")
    WqL_d = nc.dram_tensor("WqL", [128, 4, 4, 128], bf16, kind="ExternalInput")
    WahL_d = nc.dram_tensor("WahL", [128, 4, 4, 128], bf16, kind="ExternalInput")
    # fp8 vocab weights: [p, j(kc pair), tile, mc, m]
    W8L_d = nc.dram_tensor("W8L", [128, 2, 2, NMC, 128], fp8, kind="ExternalInput")
    RwL_d = nc.dram_tensor("RwL", [128, 2, 2, NMC, 128], fp8, kind="ExternalInput")
    keysT_d = nc.dram_tensor("keysT", [128, 4, 256], bf16, kind="ExternalInput")
    ZxT_d = nc.dram_tensor("ZxT", [128, 16, T, BS], bf16, kind="ExternalInput")
    c0T2_d = nc.dram_tensor("c0T2", [128, 4, BS], f32, kind="ExternalInput")
    vcol_d = nc.dram_tensor("vcol", [128, 4], bf16, kind="ExternalInput")
    bmaskT_d = nc.dram_tensor("bmaskT", [128, 2, CB], bf16, kind="ExternalInput")
    bmask2_d = nc.dram_tensor("bmask2", [4, 128], bf16, kind="ExternalInput")
    ident_d = nc.dram_tensor("identb", [128, 128], bf16, kind="ExternalInput")

    TCOLS = sum((b_ - a) * B // 2 for a, b_ in GROUPS)  # 992
    outT_d = nc.dram_tensor("outT", [VS, TCOLS], bf16, kind="ExternalOutput")
    outT_v = outT_d[:].rearrange("(m p) c -> p m c", p=128)

    colbase = []
    cb = 0
    for (t0, t1) in GROUPS:
        colbase.append(cb)
        cb += (t1 - t0) * GSIZE * BS
    assert cb == TCOLS

    with tile.TileContext(nc) as tc:
        with (
            tc.tile_pool(name="wts", bufs=1) as wts,
            tc.tile_pool(name="dram", bufs=1, space="DRAM") as dpool,
        ):
            ident_sb = wts.tile([128, 128], bf16)
            ZxT_sb = wts.tile([128, 16, T, BS], bf16)
            c0T2_sb = wts.tile([128, 4, BS], f32)
            WqL_sb = wts.tile([128, 4, 4, 128], bf16)
            keysT_sb = wts.tile([128, 4, 256], bf16)
            vcol_sb = wts.tile([128, 4], bf16)
            bmaskT_sb = wts.tile([128, 2, CB], bf16)
            bmask2_sb = wts.tile([4, 128], bf16)
            W1L_sb = wts.tile([128, 4, 16, 128], bf16)
            M2L_sb = wts.tile([128, 2, 16, 128], bf16)
            M3L_sb = wts.tile([128, 2, 4, 128], bf16)
            WahL_sb = wts.tile([128, 4, 4, 128], bf16)
            W8L_sb = wts.tile([128, 2, 2, NMC, 128], fp8)
            RwL_sb = wts.tile([128, 2, 2, NMC, 128], fp8)

            # Early-load plan: Pool takes M2L/M3L (needed by step-0 AL
            # matmuls) then RwL (needed only by the first vocab unit ~31us);
            # SP takes the step-0/1 critical ZxT-head/WqL/W1L then W8L;
            # Act keeps only its own small operands, so the first gates
            # fire as soon as z(0) exists.
            nc.gpsimd.dma_start(M2L_sb[:], M2L_d[:])
            nc.gpsimd.dma_start(M3L_sb[:], M3L_d[:])
            nc.gpsimd.dma_start(RwL_sb[:], RwL_d[:])
            nc.scalar.dma_start(c0T2_sb[:], c0T2_d[:])
            nc.scalar.dma_start(WahL_sb[:], WahL_d[:])
            nc.scalar.dma_start(keysT_sb[:], keysT_d[:])
            nc.scalar.dma_start(vcol_sb[:], vcol_d[:])
            nc.scalar.dma_start(bmaskT_sb[:], bmaskT_d[:])
            nc.scalar.dma_start(bmask2_sb[:], bmask2_d[:])
            nc.sync.dma_start(ident_sb[:], ident_d[:])
            nc.sync.dma_start(ZxT_sb[:, :, 0:6, :], ZxT_d[:, :, 0:6, :])
            nc.sync.dma_start(WqL_sb[:], WqL_d[:])
            nc.sync.dma_start(W1L_sb[:], W1L_d[:])
            nc.sync.dma_start(ZxT_sb[:, :, 6:, :], ZxT_d[:, :, 6:, :])
            nc.sync.dma_start(W8L_sb[:], W8L_d[:])

            attT_dram = []
            ag_dram = []
            for gi, (t0, t1) in enumerate(GROUPS):
                cols = (t1 - t0) * BS
                attT_dram.append(dpool.tile([H, 2, cols], fp8, name=f"attT_{gi}"))
                ag_dram.append(
                    dpool.tile([GSIZE, H, 2, cols], fp8, name=f"ag_{gi}"))

            with (
                tc.tile_pool(name="zatA", bufs=2, space="PSUM") as zpA,
                tc.tile_pool(name="zatB", bufs=2, space="PSUM") as zpB,
                tc.tile_pool(name="scrA", bufs=1, space="PSUM") as spA,
                tc.tile_pool(name="scrB", bufs=1, space="PSUM") as spB,
                tc.tile_pool(name="vps", bufs=2, space="PSUM") as vpool,
                tc.tile_pool(name="recA", bufs=2) as recA,
                tc.tile_pool(name="recB", bufs=2) as recB,
                tc.tile_pool(name="stA", bufs=2) as stA,
                tc.tile_pool(name="stB", bufs=2) as stB,
                tc.tile_pool(name="acc", bufs=2) as accpool,
                tc.tile_pool(name="asb", bufs=3) as asbpool,
                tc.tile_pool(name="stg", bufs=3) as stgpool,
            ):
                # ---------- vocab machinery ----------
                A_sb = {}
                vocab_queue = []
                evac_ctr = [0]
                stg_cur = {}

                def emit_group_gather(gi, acc):
                    t0, t1 = GROUPS[gi]
                    cols = (t1 - t0) * BS
                    # staging DMA on SP (idle mid-kernel; Act is busy with
                    # recurrence tanh); high priority so the collective
                    # fires as soon as its data is ready
                    with tc.high_priority():
                        _emit_group_gather(gi, acc, cols)

                def _emit_group_gather(gi, acc, cols):
                    nc.scalar.dma_start(
                        attT_dram[gi][:].rearrange(
                            "(kc p) kind c -> p kind kc c", p=128),
                        acc[:])
                    nc.gpsimd.collective_compute(
                        "AllGather", mybir.AluOpType.bypass,
                        replica_groups=[[0, 1, 2, 3], [4, 5, 6, 7]],
                        ins=[attT_dram[gi][:]], outs=[ag_dram[gi][:]])
                    asb = asbpool.tile([128, 2, 4, GSIZE * cols], fp8,
                                       name="A_sb")
                    A_sb[gi] = asb
                    for kind in range(2):
                        for kc in range(4):
                            nc.gpsimd.dma_start(
                                asb[:, kind, kc, :]
                                .rearrange("p (g c) -> p g c", g=GSIZE),
                                ag_dram[gi][:, 128 * kc:128 * kc + 128,
                                            kind, :]
                                .rearrange("g h c -> h g c"))

                def _vocab_unit(gi, mc):
                    t0, t1 = GROUPS[gi]
                    rows = (t1 - t0) * GSIZE * BS
                    asb = A_sb[gi]
                    vps = vpool.tile([128, 512], f32, name="vps")
                    for j in range(2):
                        nc.tensor.matmul(
                            vps[:, :rows], W8L_sb[:, j, :, mc, :],
                            asb[:, 0, 2 * j:2 * j + 2, :rows],
                            start=(j == 0), stop=False, perf_mode=DR)
                    for j in range(2):
                        nc.tensor.matmul(
                            vps[:, :rows], RwL_sb[:, j, :, mc, :],
                            asb[:, 0, 2 * j:2 * j + 2, :rows],
                            start=False, stop=False, perf_mode=DR)
                    for j in range(2):
                        nc.tensor.matmul(
                            vps[:, :rows], W8L_sb[:, j, :, mc, :],
                            asb[:, 1, 2 * j:2 * j + 2, :rows],
                            start=False, stop=(j == 1), perf_mode=DR)
                    if gi not in stg_cur or stg_cur[gi][1] != mc // 4:
                        stg_cur[gi] = (stgpool.tile([128, 4, 512], bf16,
                                                    name="stg"), mc // 4)
                    stg, mcq = stg_cur[gi]
                    if evac_ctr[0] % 2 == 0:
                        nc.scalar.copy(stg[:, mc % 4, :rows], vps[:, :rows])
                    else:
                        nc.vector.tensor_copy(stg[:, mc % 4, :rows],
                                              vps[:, :rows])
                    evac_ctr[0] += 1
                    if mc % 4 == 3 or mc == NMC - 1:
                        w = (mc % 4) + 1
                        nc.sync.dma_start(
                            outT_v[:, 4 * mcq:4 * mcq + w,
                                   colbase[gi]:colbase[gi] + rows],
                            stg[:, :w, :rows])

                def emit_vocab_unit():
                    gi, mc = vocab_queue.pop(0)
                    with tc.tile_wait_until(READY_MS[gi]):
                        _vocab_unit(gi, mc)

                def pump(n):
                    for _ in range(min(n, len(vocab_queue))):
                        emit_vocab_unit()

                # ---------- per-chain recurrence emission ----------
                chain = []
                for ci, (rec, st, zp, sp) in enumerate(
                        ((recA, stA, zpA, spA), (recB, stB, zpB, spB))):
                    bs = slice(4 * ci, 4 * ci + 4)
                    chain.append(dict(
                        rec=rec, st=st, zp=zp, sp=sp, ci=ci, bs=bs,
                        C2=c0T2_sb[:, :, bs],
                        H=None, zatt=None, z=None, att=None,
                        keysT=keysT_sb[:, :, 128 * ci:128 * ci + 128],
                        bmT=bmaskT_sb[:, ci, :],
                        bm2=bmask2_sb[:],
                    ))

                def z_init(ch, t):
                    # start of the merged z(t)/att(t-1) bank group
                    ch["zatt"] = ch["zp"].tile([128, 512], f32, name="zatt")
                    ch["z"] = ch["zatt"][:, 0:64].rearrange(
                        "p (g c b) -> p g c b", g=4, c=4)
                    ch["att"] = ch["zatt"][:, 64:80].rearrange(
                        "p (m b) -> p m b", m=4)
                    nc.tensor.matmul(
                        ch["z"], ident_sb[:], ZxT_sb[:, :, t, ch["bs"]],
                        start=True, stop=(t == 0))

                # --- per-chain stages; emission is software-pipelined
                # --- with the two chains offset by half a step so each
                # --- engine queue alternates ready work (the Tile backend
                # --- preserves per-engine program order, so emission order
                # --- decides overlap)
                def front(ch, t):
                    """gates -> H -> qT -> q copy -> zatt H-parts -> scarg -> th"""
                    rec, st, ci = ch["rec"], ch["st"], ch["ci"]
                    tga = rec.tile([128, 4, 4, 4], bf16, name="tga")
                    nc.scalar.activation(tga[:], ch["z"], AF.Tanh, scale=0.5)
                    A_t = rec.tile([128, 4, 4], f32, name="A_t")
                    B_t = rec.tile([128, 4, 4], f32, name="B_t")
                    nc.vector.scalar_tensor_tensor(
                        out=A_t[:], in0=tga[:, 2], scalar=1.0, in1=ch["C2"],
                        op0=OP.add, op1=OP.mult)
                    nc.vector.scalar_tensor_tensor(
                        out=B_t[:], in0=tga[:, 0], scalar=1.0, in1=tga[:, 1],
                        op0=OP.add, op1=OP.mult)
                    C2_new = st.tile([128, 4, 4], f32, name="C2_new")
                    nc.vector.scalar_tensor_tensor(
                        out=C2_new[:], in0=A_t[:], scalar=0.5, in1=B_t[:],
                        op0=OP.mult, op1=OP.add)
                    tc2 = rec.tile([128, 4, 4], bf16, name="tc2")
                    nc.scalar.activation(tc2[:], C2_new[:], AF.Tanh, scale=0.5)
                    H_new = st.tile([128, 4, 4], bf16, name="H_new")
                    nc.vector.scalar_tensor_tensor(
                        out=H_new[:], in0=tga[:, 3], scalar=1.0, in1=tc2[:],
                        op0=OP.add, op1=OP.mult)
                    ch["C2"] = C2_new
                    ch["H"] = H_new

                    scr = ch["sp"].tile([128, 512], f32, name="scr")
                    ch["scr"] = scr
                    qT_ps = scr[:, 0:16].rearrange("p (m b) -> p m b", m=4)
                    for m4 in range(4):
                        for kc in range(4):
                            nc.tensor.matmul(
                                qT_ps[:, m4, :], WqL_sb[:, kc, m4, :],
                                H_new[:, kc, :],
                                start=(m4 == 0 and kc == 0),
                                stop=(m4 == 3 and kc == 3))
                    qsb = rec.tile([128, 4, 4], bf16, name="qsb")
                    nc.vector.tensor_copy(qsb[:], qT_ps)

                    # z(t+1)/att(t) H-parts: ready as soon as H_new is
                    if t < T - 1:
                        z_init(ch, t + 1)
                        for m in range(16):
                            zsl = ch["z"][:, m // 4, m % 4, :]
                            for kc in range(4):
                                nc.tensor.matmul(zsl, W1L_sb[:, kc, m, :],
                                                 H_new[:, kc, :],
                                                 start=False, stop=False)
                    att_first = (t == T - 1)
                    for m4 in range(4):
                        for kc in range(4):
                            nc.tensor.matmul(
                                ch["att"][:, m4, :], WahL_sb[:, kc, m4, :],
                                H_new[:, kc, :],
                                start=(att_first and m4 == 0 and kc == 0),
                                stop=False)

                    scarg = rec.tile([128, 4, 32, 4], bf16, name="scarg")
                    th = rec.tile([128, 4, 128], bf16, name="th")
                    for hh in range(2):
                        hs = slice(2 * hh, 2 * hh + 2)
                        nc.vector.tensor_tensor(
                            out=scarg[:, hs],
                            in0=ch["keysT"][:, hs, :].rearrange(
                                "p h (s b) -> p h s b", b=4),
                            in1=qsb[:, hs].unsqueeze(2).broadcast_to(
                                [128, 2, 32, 4]),
                            op=OP.add)
                        nc.scalar.activation(
                            th[:, hs],
                            scarg[:, hs].rearrange("p h s b -> p h (s b)"),
                            AF.Tanh)
                    ch["th"] = th

                def back(ch, t):
                    """score -> softmax -> AL parts -> att evac.

                    High priority: these are the critical-path tail of the
                    recurrence step; without it the greedy engine scheduler
                    runs the other chain's queued tanh first and blocks the
                    step hand-off (~0.5us/step observed)."""
                    with tc.high_priority():
                        _back(ch, t)
                        _evac(ch, t)

                def _back(ch, t):
                    rec, ci = ch["rec"], ch["ci"]
                    scr, th = ch["scr"], ch["th"]
                    for hc in range(4):
                        nc.tensor.matmul(
                            scr[:, 16:17], th[:, hc, :],
                            vcol_sb[:, hc:hc + 1], start=(hc == 0),
                            stop=(hc == 3))
                    expT = rec.tile([128, 1], bf16, name="expT")
                    nc.scalar.activation(expT[:], scr[:, 16:17], AF.Exp)
                    nc.tensor.matmul(scr[0:4, 17:18], ch["bmT"], expT[:],
                                     start=True, stop=True)
                    rws = rec.tile([4, 1], bf16, name="rws")
                    with nc.allow_low_precision(reason="bf16 softmax scale"):
                        nc.vector.reciprocal(rws[:], scr[0:4, 17:18])
                    nc.tensor.matmul(scr[:, 18:19], ch["bm2"], rws[:],
                                     start=True, stop=True)
                    tmp_sb = rec.tile([128, 1], bf16, name="tmp_sb")
                    nc.vector.tensor_tensor(
                        out=tmp_sb[:], in0=expT[:], in1=scr[:, 18:19],
                        op=OP.mult)
                    ALT = rec.tile([128, 4], bf16, name="ALT")
                    nc.vector.tensor_tensor(
                        out=ALT[:], in0=ch["bmT"],
                        in1=tmp_sb[:].broadcast_to([128, 4]), op=OP.mult)

                    if t < T - 1:
                        for m in range(16):
                            nc.tensor.matmul(
                                ch["z"][:, m // 4, m % 4, :],
                                M2L_sb[:, ci, m, :], ALT[:], start=False,
                                stop=False)
                    for m4 in range(4):
                        nc.tensor.matmul(
                            ch["att"][:, m4, :], M3L_sb[:, ci, m4, :],
                            ALT[:], start=False, stop=(m4 == 3))

                def _evac(ch, t):
                    # quantized evac: A8 = fp8(att), Ra = fp8(att - A8).
                    # Off the critical path (consumed only at group end),
                    # so normal priority.
                    ci = ch["ci"]
                    gi = next(i for i, (a, b_) in enumerate(GROUPS)
                              if a <= t < b_)
                    t0g = GROUPS[gi][0]
                    acc = acc_of[gi]
                    c0 = 8 * (t - t0g) + 4 * ci
                    a8 = acc[:, 0, :, c0:c0 + 4]
                    nc.vector.tensor_copy(a8, ch["att"])
                    nc.vector.tensor_tensor(
                        out=acc[:, 1, :, c0:c0 + 4], in0=ch["att"],
                        in1=a8, op=OP.subtract)

                # ---------- main loop: half-step staggered pipeline ----------
                A, Bc = chain
                gathered = set()
                queued = set()
                acc_of = {}

                def ensure_acc(t):
                    gi = next(i for i, (a, b_) in enumerate(GROUPS)
                              if a <= t < b_)
                    if gi not in acc_of:
                        t0g, t1g = GROUPS[gi]
                        acc_of[gi] = accpool.tile(
                            [128, 2, 4, (t1g - t0g) * BS], fp8,
                            name="att_acc")
                    return gi

                def after_back(ch, t):
                    # when the LAST chain finishes step t of a group's end,
                    # fire the gather
                    if ch is Bc:
                        gi = next(i for i, (a, b_) in enumerate(GROUPS)
                                  if a <= t < b_)
                        if t == GROUPS[gi][1] - 1:
                            emit_group_gather(gi, acc_of[gi])
                            gathered.add(gi)

                def maybe_queue(t):
                    for gj, (a, b_) in enumerate(GROUPS):
                        if gj not in queued and gj in gathered and t >= b_ + RIPE:
                            queued.add(gj)
                            vocab_queue.extend((gj, mc) for mc in range(NMC))

                z_init(A, 0)
                z_init(Bc, 0)
                ensure_acc(0)
                front(A, 0)
                front(Bc, 0)
                back(A, 0)
                for t in range(1, T):
                    maybe_queue(t)
                    ensure_acc(t)
                    front(A, t)
                    pump(1)
                    back(Bc, t - 1)
                    after_back(Bc, t - 1)
                    pump(2)
                    front(Bc, t)
                    back(A, t)
                    after_back(A, t)
                    pump(2)
                back(Bc, T - 1)
                after_back(Bc, T - 1)

                for gj in range(len(GROUPS)):
                    if gj not in queued:
                        queued.add(gj)
                        vocab_queue.extend((gj, mc) for mc in range(NMC))
                while vocab_queue:
                    emit_vocab_unit()

    nc.compile()
    return nc


def _prep_inputs(inputs):
    dec = np.asarray(inputs["dec_input"])
    memory = np.ascontiguousarray(np.asarray(inputs["memory"], np.float32))
    h0 = np.asarray(inputs["h0"], np.float32)
    c0 = np.asarray(inputs["c0"], np.float32)
    emb = np.ascontiguousarray(np.asarray(inputs["emb"], np.float32))
    Wk = np.asarray(inputs["Wk"], np.float32)
    Wr = np.asarray(inputs["Wr"], np.float32)
    b = np.asarray(inputs["b"], np.float32)
    Wm = np.asarray(inputs["Wm"], np.float32)
    Wq = np.asarray(inputs["Wq"], np.float32)
    v = np.asarray(inputs["v"], np.float32)
    Wa = np.asarray(inputs["Wa"], np.float32)
    Wfc = np.asarray(inputs["Wfc"], np.float32)

    # gate order [i, g, f, o] (orig [i, f, g, o])
    perm = np.concatenate([np.arange(0, H), np.arange(2 * H, 3 * H),
                           np.arange(H, 2 * H), np.arange(3 * H, 4 * H)])
    Wk_p, Wr_p, b_p = Wk[:, perm], Wr[:, perm], b[perm]
    Wk_x, Wk_a = Wk_p[:E], Wk_p[E:]
    Wa_h, Wa_c = Wa[:H], Wa[H:]
    W1 = 0.5 * (Wa_h @ Wk_a + Wr_p)
    Wcg = (Wa_c @ Wk_a).copy()
    # double g-gate columns so a single tanh(0.5*z) computes all gates
    W1[:, H:2 * H] *= 2.0
    Wcg[:, H:2 * H] *= 2.0

    def lhsT(w, kchunks, mchunks):
        return np.ascontiguousarray(
            w.reshape(kchunks, 128, mchunks, 128).transpose(1, 0, 2, 3)
        ).astype(BF16)

    W1L = lhsT(W1, 4, 16)
    WqL = lhsT(0.5 * Wq, 4, 4)
    WahL = lhsT(0.5 * Wa_h, 4, 4)

    # (s,b)-ordered partition masks: partition p = s*CB + b_local
    bmaskT = np.zeros((128, 2, CB), np.float32)
    bmask2 = np.zeros((4, 128), np.float32)
    for p in range(128):
        bmaskT[p, :, p % CB] = 1.0
        bmask2[p % CB, p] = 1.0

    # fp8 vocab weights, per vocab shard (assigned in the per-core loop)
    Wfc_pad = np.zeros((H, V_PAD), np.float32)
    Wfc_pad[:, :V] = Wfc

    def fp8_pair(w16):
        w8 = w16.astype(F8)
        rw = (w16 - w8.astype(np.float32)).astype(F8)
        return w8, rw

    def lhsT8(w):  # [512, VS] -> [128, j, tile, mc, 128]
        return np.ascontiguousarray(
            w.reshape(2, 2, 128, NMC, 128).transpose(2, 0, 1, 3, 4))

    W8L_all, RwL_all = [], []
    for s in range(4):
        w16 = 16.0 * Wfc_pad[:, VS * s:VS * s + VS]
        w8, rw = fp8_pair(w16)
        W8L_all.append(lhsT8(w8))
        RwL_all.append(lhsT8(rw))

    Xall = emb[dec]
    Zx_all = Xall @ Wk_x + b_p
    Zx_all[:, 0, :] += h0 @ Wr_p
    Zx_all[:, :, H:2 * H] *= 2.0

    # (s,b) row permutation within each 4-row chain block:
    # new row r = s*CB + b  <- old row b*S + s
    sb_perm = np.empty(128, np.int64)
    for s_ in range(S):
        for b_ in range(CB):
            sb_perm[s_ * CB + b_] = b_ * S + s_

    common = dict(
        W1L=W1L, WqL=WqL, WahL=WahL,
        vcol=np.ascontiguousarray(v.reshape(4, 128).T).astype(BF16),
        bmaskT=bmaskT.astype(BF16),
        bmask2=bmask2.astype(BF16),
        identb=np.eye(128, dtype=np.float32).astype(BF16),
    )

    in_maps = []
    for g in range(NCORES):
        bsl = slice(BS * g, BS * g + BS)
        memflat = memory[bsl].reshape(BS * S, MDIM)
        # permute rows to (s,b) order per chain
        rows = np.concatenate([sb_perm, 128 + sb_perm])
        memflat_sb = memflat[rows]
        M2 = memflat_sb @ Wcg
        M3 = memflat_sb @ Wa_c
        keys = memflat_sb @ Wm
        keysT = np.ascontiguousarray(
            keys.T.reshape(4, 128, 256).transpose(1, 0, 2)).astype(BF16)
        Zx = Zx_all[bsl]
        ZxT = np.ascontiguousarray(
            Zx.transpose(2, 1, 0).reshape(16, 128, T, BS)
            .transpose(1, 0, 2, 3)).astype(BF16)
        c0T2 = np.ascontiguousarray(
            (2.0 * c0[bsl]).T.reshape(4, 128, BS).transpose(1, 0, 2)
        ).astype(np.float32)
        m = dict(common)
        m.update(
            M2L=lhsT(M2, 2, 16),
            M3L=lhsT(M3, 2, 4),
            keysT=keysT,
            ZxT=ZxT,
            c0T2=c0T2,
            W8L=W8L_all[g % 4],
            RwL=RwL_all[g % 4],
        )
        in_maps.append(m)
    return in_maps


def kernel(**inputs):
    from concourse.bass_utils import run_bass_kernel_spmd

    if "nc" not in _CACHE:
        _CACHE["nc"] = _build_program()
    nc = _CACHE["nc"]

    in_maps = _prep_inputs(inputs)
    res = run_bass_kernel_spmd(nc, in_maps, list(range(NCORES)))

    bfc = np.asarray(inputs["bfc"], np.float32)
    full = np.empty((B, T, V_PAD), np.float32)
    for k in range(NCORES):
        O = np.asarray(res.results[k]["outT"]).astype(np.float32)
        half, shard = k // 4, k % 4
        cb = 0
        for (t0, t1) in GROUPS:
            tn = t1 - t0
            blk = O[:, cb:cb + tn * GSIZE * BS].reshape(VS, GSIZE, tn, BS)
            # column order: (g within replica group, t, b)
            full[32 * half:32 * half + 32, t0:t1,
                 VS * shard:VS * shard + VS] = (
                blk.transpose(1, 3, 2, 0).reshape(32, tn, VS))
            cb += tn * GSIZE * BS
    out = full[:, :, :V] * (1.0 / 16.0) + bfc[None, None, :]
    return np.ascontiguousarray(out.astype(np.float32))
